# revision 1
# baseline (speedup 1.0000x reference)
"""BayesianLinear (y = x @ (mu + softplus(rho) * eps).T + bias) on 8 TRN2 cores.

Column-parallel sharding: each core owns OUT_F/8 = 512 output features.

Host-side prep is pure layout/precision staging (no reference math):
  - x is cast to bf16 and pre-tiled into the SBUF layout the TensorEngine
    needs for its stationary operand: x_t[bt, pi, po, bi] = x[bt*128+bi,
    po*128+pi]. (An fp8-e4m3 DoubleRow variant of the trailing K-blocks
    was measured: walrus/TRN2 ran the DoubleRow matmuls at 1 elem/cycle
    — no ALU win — AND the presence of fp8 matmuls downclocked the PE
    2.4->2.0 GHz for the whole run, a 26us net loss. All-bf16 it is.)
  - weight_mu/rho/eps shards are transposed to [in_f, o_sh], tiled per
    128-row K-block, and PACKED into one bf16-typed tensor per K-block
    pair (mu bf16 | eps bf16 | rho fp16-bits) so W^T construction costs
    a single DMA per pair. rho ships as fp16 because softplus amplifies
    its quantization ~3x.

Device per core:
  1. Bias row (tiny DMAs first on the sync queue) and 16 packed param
     DMAs interleaved with the 8-tile group's x chunk loads in
     hand-picked order (pk pairs lead; W^T pair j always lands just
     ahead of its consumption). Each DMA_DIRECT2D trigger costs ~0.7us
     of issuing-engine time, so the group's later x chunks are triggered
     from the scalar engine's HWDGE queue in the slack between softplus
     ops — one queue's trigger rate alone caps phase-1 x delivery below
     the PE's consumption rate. (The GPSIMD SWDGE queue the packs used
     to ride is ~9us slower to first byte, which idled the PE.)
  2. softplus(rho) = Ln(1 + Exp(rho)) on ACT, mul/add on DVE writing
     bf16 into the resident W^T tile [128, 32, 512].
  3. PE program order: 17 warmup K=1 matmuls (HAM clock ramp + cover of
     the construction latency), bias broadcast (ones.T @ bias_bf) and
     its eviction (frees the 8th PSUM bank), then a split-K group: tiles
     0-7 accumulate k 0..15 k-interleaved (paced against construction),
     park bias-pre-added fp32 partials in SBUF, tiles 8-11 stream their
     first halves at full speed off the resident W^T, then both sets
     finish k 16..31 — 12 tiles of phase-1 PE work (~99us) against the
     ~24.6 MB phase-1 delivery keeps the PE the bottleneck even when
     HBM delivers only ~280 GB/s (observed range 260-410 plus whole-run
     2.4->2.0 GHz PE downclock periods, both outside kernel control).
     Remaining 52 tiles stream one PSUM bank each: 32 accumulating bf16
     matmuls, DVE eviction fused with the partial/bias add, y out on the
     scalar HWDGE queue (a y trigger waits on its tile's eviction and
     would head-of-line block x prefetch on the sync queue).
"""

import numpy as np
import ml_dtypes

import concourse.bacc as bacc
import concourse.mybir as mybir
import concourse.tile as tile
from concourse.bass_utils import run_bass_kernel_spmd

BATCH = 8192
IN_F = 4096
OUT_F = 4096
N_CORES = 8
P = 128
KF8 = 0  # fp8 DoubleRow K-blocks: disabled (downclocks the PE, no ALU win)

_NC_CACHE = {}


def build_nc(batch=BATCH, in_f=IN_F, o_sh=OUT_F // N_CORES, kf8=KF8):
    KB = in_f // P  # K-blocks of 128 along the contraction dim
    BT = batch // P  # 128-row output tiles
    K2 = 2 if KB % 2 == 0 else 1  # K-blocks per construction step
    NPAIR = KB // K2
    kbf = KB - kf8  # leading bf16 K-blocks
    assert kf8 % K2 == 0 and kbf % K2 == 0

    nc = bacc.Bacc(
        "TRN2",
        target_bir_lowering=False,
        debug=False,
        enable_asserts=False,
        num_devices=N_CORES,
    )
    bf16 = mybir.dt.bfloat16
    f16 = mybir.dt.float16
    f8 = mybir.dt.float8e4
    f32 = mybir.dt.float32

    xb = nc.declare_dram_parameter("x_bf", [BT, P, kbf, P], bf16, isOutput=False)
    x8 = (
        nc.declare_dram_parameter("x_f8", [BT, P, kf8, P], f8, isOutput=False)
        if kf8
        else None
    )
    wpk = nc.declare_dram_parameter(
        "wpk_t", [NPAIR, P, K2, 3 * o_sh], bf16, isOutput=False
    )
    bpk = nc.declare_dram_parameter("bias_pk", [1, 3 * o_sh], f32, isOutput=False)
    y = nc.declare_dram_parameter("y", [batch, o_sh], f32, isOutput=True)

    act_exp = mybir.ActivationFunctionType.Exp
    act_ln = mybir.ActivationFunctionType.Ln

    GROUP = 8
    N_WARM = 17

    with tile.TileContext(nc) as tc:
        with (
            tc.tile_pool(name="const", bufs=1) as const,
            tc.tile_pool(name="wcons", bufs=3) as wcons,
            tc.tile_pool(name="xin", bufs=13) as xin,
            tc.tile_pool(name="part", bufs=12) as part,
            tc.tile_pool(name="yout", bufs=2) as yout,
            tc.tile_pool(name="psum", bufs=8, space="PSUM") as psum_pool,
        ):
            bias_sb = const.tile([P, o_sh], f32, tag="bias_sb")
            bias_bf = const.tile([1, o_sh], bf16, tag="bias_bf")
            ones = const.tile([1, P], bf16, tag="ones")
            nc.vector.memset(ones[:], 1.0)
            wones = const.tile([1, o_sh], bf16, tag="wones")
            nc.vector.memset(wones[:], 1.0)

            # Bias inputs ride the sync queue ahead of everything (6 KiB,
            # one packed partition-0 [1, 3*o_sh] DMA: mu | rho | eps).
            b_all = const.tile([1, 3 * o_sh], f32, tag="b_all")
            nc.sync.dma_start(out=b_all[:], in_=bpk[:])
            b_mu = b_all[:, 0:o_sh]
            b_rho = b_all[:, o_sh : 2 * o_sh]
            b_eps = b_all[:, 2 * o_sh : 3 * o_sh]
            b_sp = const.tile([1, o_sh], f32, tag="b_sp")
            nc.scalar.activation(b_sp[:], b_rho, act_exp)
            nc.scalar.activation(b_sp[:], b_sp[:], act_ln, bias=1.0)
            nc.vector.tensor_mul(out=b_sp[:], in0=b_sp[:], in1=b_eps)
            nc.vector.tensor_add(out=bias_bf[:], in0=b_sp[:], in1=b_mu)

            # ---- sync-queue DMA program: wpk pairs interleaved with the
            # group's x chunk loads so delivery tracks consumption order.
            # Each pair's construction ops (ACT softplus, DVE mul/add) are
            # emitted right after its DMA so the 3-deep pk ring's reuse
            # dependencies are in place before the ring wraps. The DVE
            # add's output dtype does the fp32->e4m3 rounding for the
            # fp8 blocks.
            WT_bf = const.tile([P, kbf, o_sh], bf16, tag="WT_bf")
            WT_f8 = const.tile([P, kf8, o_sh], f8, tag="WT_f8") if kf8 else None
            EXTRA = 4  # tiles 8-11: full-speed half-passes between A and B
            HALF = kbf // 2
            xbs = []
            x8s = []
            for bt in range(GROUP + EXTRA):
                xbs.append(xin.tile([P, kbf, P], bf16, tag="xT", name=f"xTb_g{bt}"))
                if kf8:
                    x8s.append(
                        xin.tile([P, kf8, P], f8, tag="x8", name=f"xT8_g{bt}")
                    )

            def emit_pair(j):
                pk = wcons.tile([P, K2, 3 * o_sh], bf16, tag="pk")
                nc.sync.dma_start(out=pk[:], in_=wpk[j])
                mu_t = pk[:, :, 0:o_sh]
                eps_t = pk[:, :, o_sh : 2 * o_sh]
                rho_t = pk[:, :, 2 * o_sh : 3 * o_sh].bitcast(f16)
                sp_t = wcons.tile([P, K2, o_sh], f32, tag="sp")
                nc.scalar.activation(sp_t[:], rho_t[:], act_exp)
                nc.scalar.activation(sp_t[:], sp_t[:], act_ln, bias=1.0)
                nc.vector.tensor_mul(out=sp_t[:], in0=sp_t[:], in1=eps_t[:])
                if j * K2 < kbf:
                    out_sl = WT_bf[:, j * K2 : (j + 1) * K2, :]
                else:
                    jf = j * K2 - kbf
                    out_sl = WT_f8[:, jf : jf + K2, :]
                nc.vector.tensor_add(out=out_sl, in0=sp_t[:], in1=mu_t[:])

            # Interleave: pk pairs lead (longest latency chain), the first
            # x chunk rides the sync queue, later chunks are triggered by
            # the scalar engine's HWDGE queue — each DMA_DIRECT2D trigger
            # costs ~0.7us of issuing-engine time, so 32 group-chunk
            # triggers on one queue would cap x delivery below the PE's
            # consumption rate. Emission order == per-queue issue order.
            NCH = 4
            bounds = [round(kbf * c / NCH) for c in range(NCH + 1)]
            XC_ENG = [nc.sync, nc.scalar, nc.scalar, nc.scalar]
            order = []
            order += [("pk", 0), ("pk", 1), ("xc", 0)]
            order += [("xc", 1)]
            order += [("pk", 2), ("pk", 3), ("pk", 4), ("pk", 5)]
            order += [("xc", 2)]
            order += [("pk", 6), ("pk", 7)]
            order += [("xe", 0)]  # extras' first halves, needed when pass A2 starts
            order += [("pk", 8), ("pk", 9)]
            order += [("xc", 3)]
            order += [("pk", j) for j in range(10, NPAIR)]
            order += [("xe", 1)]  # extras' second halves, needed at pass B2
            if kf8:
                order.append(("x8", 0))
            for kind, idx in order:
                if kind == "pk":
                    emit_pair(idx)
                elif kind == "xc":
                    ks = slice(bounds[idx], bounds[idx + 1])
                    for i in range(GROUP):
                        XC_ENG[idx].dma_start(out=xbs[i][:, ks, :], in_=xb[i, :, ks, :])
                elif kind == "xe":
                    ks = slice(idx * HALF, (idx + 1) * HALF)
                    for e in range(GROUP, GROUP + EXTRA):
                        nc.sync.dma_start(out=xbs[e][:, ks, :], in_=xb[e, :, ks, :])
                else:
                    for i in range(GROUP):
                        nc.sync.dma_start(out=x8s[i][:], in_=x8[i])

            # ---- PE program: warmup (HAM ramp, covers construction
            # latency), bias broadcast, then the matmul stream.
            warm_ps = psum_pool.tile([P, o_sh], f32, tag="ps", name="warm_ps")
            for w in range(N_WARM):
                nc.tensor.matmul(warm_ps[:], lhsT=ones[:], rhs=wones[:])
            bias_ps = psum_pool.tile([P, o_sh], f32, tag="ps", name="bias_ps")
            nc.tensor.matmul(bias_ps[:], lhsT=ones[:], rhs=bias_bf[:])
            nc.vector.tensor_copy(out=bias_sb[:], in_=bias_ps[:])

            def emit_tile_mms(ps, xbf_t, xf8_t):
                for k in range(kbf):
                    nc.tensor.matmul(
                        ps[:],
                        lhsT=xbf_t[:, k, :],
                        rhs=WT_bf[:, k, :],
                        start=(k == 0),
                        stop=(k == kbf - 1 and not kf8),
                    )
                for j in range(kf8 // 2):
                    nc.tensor.matmul(
                        ps[:],
                        lhsT=xf8_t[:, 2 * j : 2 * j + 2, :],
                        rhs=WT_f8[:, 2 * j : 2 * j + 2, :],
                        start=False,
                        stop=(j == kf8 // 2 - 1),
                        perf_mode=mybir.MatmulPerfMode.DoubleRow,
                    )

            def body_tail(ps, bt):
                y_sb = yout.tile([P, o_sh], f32, tag="y_sb")
                nc.vector.tensor_add(out=y_sb[:], in0=ps[:], in1=bias_sb[:])
                # y rides the scalar HWDGE queue: a y trigger waits on its
                # tile's eviction, and on the sync queue that wait would
                # head-of-line block the x prefetch stream behind it.
                nc.scalar.dma_start(out=y[bt * P : (bt + 1) * P, :], in_=y_sb[:])

            # Split-K group: PSUM's 8 banks cap the k-interleaved width,
            # but half-K passes let 12 tiles share phase 1 — pass A (tiles
            # 0-7, k 0..HALF) paces the PE against W^T construction, its
            # fp32 partials (with bias pre-added, so eviction cost is
            # unchanged) park in SBUF; tiles 8-11 then stream their first
            # halves at full speed off the resident W^T while pairs
            # HALF.. construct; pass B finishes tiles 0-7, then the
            # extras. Phase-1 PE work ~99us vs ~24.6 MB delivered keeps
            # the PE the bottleneck even when HBM delivers only ~280 GB/s
            # (observed run-to-run range 260-410).
            assert not kf8
            pss = [
                psum_pool.tile([P, o_sh], f32, tag="ps", name=f"ps_a{bt}")
                for bt in range(GROUP)
            ]
            for k in range(HALF):
                for i in range(GROUP):
                    nc.tensor.matmul(
                        pss[i][:],
                        lhsT=xbs[i][:, k, :],
                        rhs=WT_bf[:, k, :],
                        start=(k == 0),
                        stop=(k == HALF - 1),
                    )
            parts = []
            for i in range(GROUP):
                pa = part.tile([P, o_sh], f32, tag="pA", name=f"pA_{i}")
                nc.vector.tensor_add(out=pa[:], in0=pss[i][:], in1=bias_sb[:])
                parts.append(pa)
            # A2: extras' first halves at full speed (W^T 0..HALF resident)
            for e in range(GROUP, GROUP + EXTRA):
                ps = psum_pool.tile([P, o_sh], f32, tag="ps", name=f"ps_a{e}")
                for k in range(HALF):
                    nc.tensor.matmul(
                        ps[:],
                        lhsT=xbs[e][:, k, :],
                        rhs=WT_bf[:, k, :],
                        start=(k == 0),
                        stop=(k == HALF - 1),
                    )
                pa = part.tile([P, o_sh], f32, tag="pA", name=f"pA_{e}")
                nc.vector.tensor_add(out=pa[:], in0=ps[:], in1=bias_sb[:])
                parts.append(pa)
            # B: tiles 0-7 second halves (k-interleaved; W^T is resident or
            # lands well ahead now that pass A2 shifted consumption +14us)
            psb = [
                psum_pool.tile([P, o_sh], f32, tag="ps", name=f"ps_b{bt}")
                for bt in range(GROUP)
            ]
            for k in range(HALF, kbf):
                for i in range(GROUP):
                    nc.tensor.matmul(
                        psb[i][:],
                        lhsT=xbs[i][:, k, :],
                        rhs=WT_bf[:, k, :],
                        start=(k == HALF),
                        stop=(k == kbf - 1),
                    )
            def split_tail(ps, pa, bt):
                y_sb = yout.tile([P, o_sh], f32, tag="y_sb")
                nc.vector.tensor_add(out=y_sb[:], in0=ps[:], in1=pa[:])
                nc.scalar.dma_start(out=y[bt * P : (bt + 1) * P, :], in_=y_sb[:])
            for i in range(GROUP):
                split_tail(psb[i], parts[i], i)
            # B2: extras' second halves at full speed
            for e in range(GROUP, GROUP + EXTRA):
                ps = psum_pool.tile([P, o_sh], f32, tag="ps", name=f"ps_b{e}")
                for k in range(HALF, kbf):
                    nc.tensor.matmul(
                        ps[:],
                        lhsT=xbs[e][:, k, :],
                        rhs=WT_bf[:, k, :],
                        start=(k == HALF),
                        stop=(k == kbf - 1),
                    )
                split_tail(ps, parts[e], e)

            # ---- remaining tiles stream one PSUM bank each
            for bt in range(GROUP + EXTRA, BT):
                xbf_t = xin.tile([P, kbf, P], bf16, tag="xT")
                nc.sync.dma_start(out=xbf_t[:], in_=xb[bt])
                xf8_t = None
                if kf8:
                    xf8_t = xin.tile([P, kf8, P], f8, tag="x8")
                    nc.sync.dma_start(out=xf8_t[:], in_=x8[bt])
                ps = psum_pool.tile([P, o_sh], f32, tag="ps")
                emit_tile_mms(ps, xbf_t, xf8_t)
                body_tail(ps, bt)

    # Skip bacc's pre-placed InstLoadActFuncSet: on large graphs walrus's
    # parallel-pass fork can separate the hoisted load from its activations
    # ("No Act func set exist for this instruction"); walrus's own lower_act
    # placement handles forked subgraphs correctly.
    nc.insert_act_table_loads = lambda: None
    nc.compile()
    return nc


def _prep_x(x, kf8=KF8):
    """[batch, in_f] fp32 -> (bf16 tiled [BT, 128, KB-kf8, 128],
    e4m3 tiled [BT, 128, kf8, 128] or None) with x_t[bt, pi, po, bi] =
    x[bt*128 + bi, po*128 + pi]."""
    batch, in_f = x.shape
    kcut = in_f - kf8 * P
    xbf = x[:, :kcut].astype(ml_dtypes.bfloat16)
    xbf = xbf.reshape(batch // P, P, kcut // P, P)  # [bt, bi, po, pi]
    xbf = np.ascontiguousarray(xbf.transpose(0, 3, 2, 1))  # [bt, pi, po, bi]
    if not kf8:
        return xbf, None
    xf8 = x[:, kcut:].astype(ml_dtypes.float8_e4m3)
    xf8 = xf8.reshape(batch // P, P, kf8, P)
    xf8 = np.ascontiguousarray(xf8.transpose(0, 3, 2, 1))
    return xbf, xf8


def _tile_w(w, dtype):
    """[o_sh, in_f] -> tiled [KB, 128, o_sh] with w_t[k, pi, o] = w[o, k*128 + pi]."""
    o_sh, in_f = w.shape
    return np.ascontiguousarray(w.T.reshape(in_f // P, P, o_sh)).astype(dtype)


def _prep_wpk(wmu, wrho, weps):
    """Pack mu (bf16), eps (bf16), rho (fp16 bits viewed as bf16) into one
    bf16-typed [KB/K2, 128, K2, 3*o_sh] tensor — one DMA per K2 K-blocks."""
    mu = _tile_w(wmu, ml_dtypes.bfloat16)
    eps = _tile_w(weps, ml_dtypes.bfloat16)
    rho = _tile_w(wrho, np.float16).view(ml_dtypes.bfloat16)
    pk = np.concatenate([mu, eps, rho], axis=2)  # [KB, P, 3*o_sh]
    kb, p, f = pk.shape
    k2 = 2 if kb % 2 == 0 else 1
    pk = pk.reshape(kb // k2, k2, p, f).transpose(0, 2, 1, 3)
    return np.ascontiguousarray(pk)


def make_in_maps(x, weight_mu, weight_rho, bias_mu, bias_rho, weight_eps, bias_eps):
    o_sh = OUT_F // N_CORES
    x_bf, x_f8 = _prep_x(np.asarray(x, dtype=np.float32))
    wmu = np.asarray(weight_mu, dtype=np.float32)
    wrho = np.asarray(weight_rho, dtype=np.float32)
    weps = np.asarray(weight_eps, dtype=np.float32)
    bpk = np.stack(
        [
            np.asarray(bias_mu, dtype=np.float32),
            np.asarray(bias_rho, dtype=np.float32),
            np.asarray(bias_eps, dtype=np.float32),
        ]
    )  # [3, OUT_F]

    in_maps = []
    for c in range(N_CORES):
        rs = slice(c * o_sh, (c + 1) * o_sh)
        im = {
                "x_bf": x_bf,
                "wpk_t": _prep_wpk(wmu[rs], wrho[rs], weps[rs]),
                "bias_pk": np.ascontiguousarray(bpk[:, rs].reshape(1, -1)),
        }
        if x_f8 is not None:
            im["x_f8"] = x_f8
        in_maps.append(im)
    return in_maps


def kernel(x, weight_mu, weight_rho, bias_mu, bias_rho, weight_eps, bias_eps):
    o_sh = OUT_F // N_CORES
    key = (x.shape, o_sh)
    if key not in _NC_CACHE:
        _NC_CACHE[key] = build_nc(x.shape[0], x.shape[1], o_sh)
    nc = _NC_CACHE[key]

    in_maps = make_in_maps(
        x, weight_mu, weight_rho, bias_mu, bias_rho, weight_eps, bias_eps
    )
    res = run_bass_kernel_spmd(nc, in_maps, core_ids=list(range(N_CORES)))
    return np.concatenate([res.results[c]["y"] for c in range(N_CORES)], axis=1)

